# revision 34
# baseline (speedup 1.0000x reference)
"""Trainium2 Bass kernel for nn_C_Aggregation_24807731101830.

Patch-embed conv (stride 16 = kernel 16) + sequential Gauss-Seidel-like
index-update scan over a flattened 34x34 grid, batch-sharded over 8 cores.

v2 (fp16 + engine rebalance):
  - all on-chip data fp16 (DVE 2x tensor_tensor, half DMA traffic), conv
    matmuls on fp16 inputs (full PE rate), PSUM accumulation fp32.
  - row scan in multiply-form: state = (d0 + state) * M, M = [1, .125*31, 0]
    per 33-col segment, folding the reference's *1/8 into the scan.
  - buf holds ONLY original conv values; scan rows go to DRAM directly from
    the per-row scan tiles (30 small DMAs over 4 per-octet base-region DMAs
    on the same queue, so DRAM write order is FIFO-correct). No writeback.
  - each scan is split: segments 0-5 on DVE, 6-11 on GpSimd (parallel).
  - inputs DMA'd from the ACT DGE queue (outputs own the SP queue), with a
    small prefetch (bias, wT, both batches' first column-quarter) so the
    first conv octet starts ~3us in.
"""
import sys
import types
import numpy as np

import concourse.mybir as mybir
from concourse import bass, tile
from concourse.bass_utils import run_bass_kernel_spmd
from contextlib import ExitStack

F32 = mybir.dt.float32
F16 = mybir.dt.float16
AOP = mybir.AluOpType
IDENT = mybir.ActivationFunctionType.Identity

N_CORES = 8
B_LOC = 2            # batches per core
CG = 6               # channel groups of 128
NBG = B_LOC * CG     # 12 scan lane-groups
Q34 = 1156           # 34*34
QF = NBG * Q34       # buf free size per partition

# GpSimd/Pool only implements the TensorTensor opcode family; the
# TensorScalarPtr family (tensor_tensor_scan, tensor_scalar) fails the
# walrus per-engine ISA check, so those stay on DVE/ACT.
POOL_SCAN = False
POOL_BORDERS = False

LAST_EXEC_NS = None


def _install_ntff_hook():
    try:
        import trn_agent_boot.trn_boot as tb
        mod = types.ModuleType("antenv.axon_hooks")
        holder = [None]
        mod.set_axon_ntff_profile_hook = lambda h: holder.__setitem__(0, h)
        mod.get_axon_ntff_profile_hook = lambda: holder[0]
        sys.modules["antenv.axon_hooks"] = mod
        import antenv
        antenv.axon_hooks = mod
        mod.set_axon_ntff_profile_hook(
            tb._ntff_profile_via_ctypes('/opt/axon/libaxon_pjrt.so'))
        return True
    except Exception:
        return False


def _split_sp_multiwaits(nc):
    """walrus for gen3 rejects >1 sync-wait on several instruction structs
    (TPB_CTRL, S3_LW, ...); hoist extra waits onto single-wait NOPs placed
    just before, on the same engine queue (semantically equivalent)."""
    cnt = 0
    for f in nc.m.functions:
        for blk in f.blocks:
            insts = blk.instructions
            i = 0
            while i < len(insts):
                inst = insts[i]
                si = getattr(inst, 'sync_info', None)
                if (getattr(inst, 'engine', None) is not None
                        and si is not None and si.on_wait and len(si.on_wait) > 1):
                    waits = list(si.on_wait)
                    new = []
                    for w in waits[:-1]:
                        nop = mybir.InstNoOp(name=f"mwfix-{inst.name}-{cnt}",
                                             ins=[], outs=[])
                        cnt += 1
                        nop.engine = inst.engine
                        nop.sync_info = mybir.SyncInfo(on_wait=[w], on_update=[])
                        new.append(nop)
                    inst.sync_info = mybir.SyncInfo(
                        on_wait=[waits[-1]], on_update=list(si.on_update or []))
                    insts[i:i] = new
                    i += len(new)
                i += 1
    return cnt


# rows of the scan that become runnable after each conv part (a part
# scattering up to grid row G covers flat < 34(G+1)+33; row i reads taps
# up to flat 32i+65). Octet 0 is split into two 4-row quads so the chain
# starts after a quarter of the first conv work.
PIECES = [(1, 3), (4, 7), (8, 16), (17, 24), (25, 30)]
# output DMA chunks of buf, flat [lo, hi), each ready after the given row
# (row r+1 writes from flat 32(r+1)+1); the never-scanned tail [993,1156)
# goes out early, right after octet 3's scatter (marker row -3)
CHUNKS = [(0, 512, 15), (512, 737, 22), (993, 1156, -3),
          (737, 897, 27), (897, 961, 29), (961, 993, 30)]


def _build():
    nc = bass.Bass("TRN2", target_bir_lowering=False)
    xP_d = nc.declare_dram_parameter("xP", [768, B_LOC, 1024], F16, isOutput=False)
    wT_d = nc.declare_dram_parameter("wT", [768, 768], F16, isOutput=False)
    bias_d = nc.declare_dram_parameter("bias", [768], F32, isOutput=False)
    xf_d = nc.declare_dram_parameter("xf", [B_LOC, 768, Q34], F16, isOutput=True)
    xfr = xf_d.rearrange("b (g p) q -> p b g q", p=128)

    with tile.TileContext(nc) as tc, ExitStack() as ctx:
        sb = ctx.enter_context(tc.tile_pool(name="sb", bufs=1))
        ps = ctx.enter_context(tc.tile_pool(name="ps", bufs=4, space="PSUM"))
        pbpool = ctx.enter_context(tc.tile_pool(name="pb", bufs=2))

        # ---- preload the ACT function table before anything else (the
        #      first real ACTIVATE otherwise eats a 1.3us table load) ----
        scr = sb.tile([128, 8], F16, tag="scr")
        nc.scalar.activation(scr[:], scr[:], IDENT)

        # ---- input loads: bias/weights + batch-1 x on the ACT DGE queue,
        #      batch-0 x on SP; both batches' first quarter go out first ----
        biast = sb.tile([128, 6], F32, tag="bias")
        nc.scalar.dma_start(biast[:], bias_d.rearrange("(a p) -> p a", p=128))
        wt = sb.tile([128, 6, 768], F16, tag="wt")
        wTr = wT_d.rearrange("(a p) c -> p a c", p=128)
        for h in range(2):
            nc.scalar.dma_start(wt[:, 3 * h:3 * h + 3, :],
                                wTr[:, 3 * h:3 * h + 3, :])
        xpt = sb.tile([128, 6, B_LOC * 1024], F16, tag="xpt")
        xPr = xP_d.rearrange("(a p) b q -> p a b q", p=128)
        xpt4 = xpt[:].rearrange("p a (b q) -> p a b q", b=B_LOC)
        # first quarter split into 128-col eighths so the first conv quad's
        # columns land as early as possible
        for q0, q1 in [(0, 128), (128, 256), (256, 512), (512, 768),
                       (768, 1024)]:
            for b in range(B_LOC):
                eng = nc.sync if b == 0 else nc.scalar
                eng.dma_start(xpt4[:, :, b:b + 1, q0:q1],
                              xPr[:, :, b:b + 1, q0:q1])

        # ---- constants ----
        mmask = sb.tile([128, NBG * 33], F16, tag="mmask")
        nc.vector.memset(mmask[:], 0.125)
        mm3 = mmask[:].rearrange("p (g c) -> p g c", g=NBG)
        nc.vector.memset(mm3[:, :, 0:1], 1.0)
        nc.vector.memset(mm3[:, :, 32:33], 0.0)
        zt = sb.tile([128, 64], F16, tag="zt")
        nc.vector.memset(zt[:], 0.0)

        # ~3us of dummy matmuls while the inputs stream in: ramps the PE
        # p-state (0.65 -> 2.4 GHz needs ~3us of continuous execution) so
        # the first real conv part runs at speed
        warmps = ps.tile([128, 64], F32, tag="warm", name="warmps", bufs=1)
        for _ in range(20):
            nc.tensor.matmul(warmps[:], lhsT=mmask[:, 0:128],
                             rhs=mmask[:, 0:64], start=True, stop=True)

        # ---- buf: original conv values only, f = bg*1156 + flat ----
        buf = sb.tile([128, QF], F16, tag="buf")
        buf3 = buf[:].rearrange("p (bg q) -> p bg q", bg=NBG)
        buf4 = buf[:].rearrange("p (bg gi gj) -> p bg gi gj", bg=NBG, gi=34)

        # ---- scratch tiles for the scan chain (permanent; the chain is
        #      sequential so reuse is safe, s rotates 4-deep for the DMAs) ----
        ua = sb.tile([128, NBG * 31], F16, tag="ua")
        uav = ua[:].rearrange("p (g c) -> p g c", g=NBG)
        ub = sb.tile([128, NBG * 31], F16, tag="ub")
        ubv = ub[:].rearrange("p (g c) -> p g c", g=NBG)
        d0t = [sb.tile([128, NBG * 33], F16, tag=f"d0_{k}", name=f"d0_{k}")
               for k in range(2)]
        st = [sb.tile([128, NBG * 33], F16, tag=f"s_{k}", name=f"s_{k}")
              for k in range(8)]
        for k in range(2):   # reset slots must be finite once and forever
            nc.vector.memset(
                d0t[k][:].rearrange("p (g c) -> p g c", g=NBG)[:, :, 32:33], 0.0)

        # ---- borders = bias. On DVE (idle during the fill anyway): on ACT
        #      they clog the FIFO ahead of the first scatters ----
        for b in range(B_LOC):
            for m in range(CG):
                bg = b * CG + m
                bcol = biast[:, m:m + 1]
                views = [buf3[:, bg, 0:35],
                         buf3[:, bg:bg + 1, 67:67 + 34 * 31].rearrange(
                             "p o (r t) -> p (o r) t", t=34)[:, :, 0:2],
                         buf3[:, bg, 1121:1156]]
                zins = [zt[:, 0:35],
                        zt[:, 0:62].rearrange("p (r t) -> p r t", t=2),
                        zt[:, 0:35]]
                for v, z in zip(views, zins):
                    nc.vector.tensor_scalar(v, z, bcol, None, op0=AOP.add)

        def band_view(base, nrows):
            return buf3[:, :, base:base + 32 * nrows].rearrange(
                "p g (r t) -> p r g t", t=32)[:, :, :, 0:31]

        pb_piece = {}

        def emit_band(piece):
            """UNSCALED 4-tap of orig for rows i0..i1 (+ col-31 fix: the
            j=31 3-tap tap orig(flat 32i) that the zeroed s_prev col 32
            cannot supply)."""
            i0, i1 = PIECES[piece]
            nr = i1 - i0 + 1
            base = 32 * i0 + 2
            pb = pbpool.tile([128, nr * NBG * 31], F16, tag="PB",
                             name=f"pb_{piece}")
            pb4 = pb[:].rearrange("p (r g j) -> p r g j", r=nr, g=NBG)
            eng0 = nc.vector if piece == 0 else nc.gpsimd  # piece 0 is fill
            eng0.tensor_tensor(pb4, band_view(base, nr),
                               band_view(base + 30, nr), AOP.add)
            tmp = pbpool.tile([128, nr * NBG * 31], F16, tag="PTMP",
                              name=f"ptmp_{piece}")
            tmp4 = tmp[:].rearrange("p (r g j) -> p r g j", r=nr, g=NBG)
            nc.vector.tensor_tensor(tmp4, band_view(base + 31, nr),
                                    band_view(base + 32, nr), AOP.add)
            nc.vector.tensor_tensor(pb[:], pb[:], tmp[:], AOP.add)
            f0 = i0 if i0 > 1 else 2
            if f0 <= i1:
                nfix = i1 - f0 + 1
                fix_dst = pb4[:, f0 - i0:, :, 30:31]
                fix_src = buf3[:, :, 32 * f0:32 * f0 + 32 * nfix].rearrange(
                    "p g (r t) -> p r g t", t=32)[:, :, :, 0:1]
                nc.vector.scalar_tensor_tensor(
                    fix_dst, fix_src, 1.0, fix_dst, AOP.mult, AOP.add)
            pb_piece[piece] = (pb, i0)

        s_prev_box = [None]
        HALF = NBG * 33 // 2

        def emit_row(i):
            qi = 32 * i
            piece = next(p for p, (a, b) in enumerate(PIECES) if a <= i <= b)
            pb, i0 = pb_piece[piece]
            pbr = pb[:].rearrange("p (r g j) -> p r g j",
                                  r=PIECES[piece][1] - i0 + 1,
                                  g=NBG)[:, i - i0, :, :]
            if s_prev_box[0] is None:
                sp = buf3[:, :, 0:33]          # row 0 = orig
            else:
                sp = s_prev_box[0][:].rearrange("p (g c) -> p g c", g=NBG)
            # d0[j] = sp[j-1] + sp[j] + sp[j+1] + P4[j]   (j = 1..31)
            nc.vector.tensor_tensor(uav, sp[:, :, 0:31], sp[:, :, 2:33],
                                    AOP.add)
            nc.vector.tensor_tensor(ubv, sp[:, :, 1:32], pbr, AOP.add)
            d0 = d0t[i % 2]
            d3 = d0[:].rearrange("p (g c) -> p g c", g=NBG)
            nc.vector.tensor_tensor(d3[:, :, 1:32], uav, ubv, AOP.add)
            # col 0 seeds the segment (M=1); col 32 stays 0 from init.
            # The seed stays on DVE: on ACT it queues behind octet scatter
            # bursts and stalls the scan chain at piece boundaries.
            nc.vector.tensor_scalar(d3[:, :, 0:1], buf3[:, :, qi:qi + 1],
                                    1.0, None, op0=AOP.mult)
            s_cur = st[i % 8]
            if POOL_SCAN:
                nc.vector.tensor_tensor_scan(
                    s_cur[:, 0:HALF], d0[:, 0:HALF], mmask[:, 0:HALF], 0.0,
                    AOP.add, AOP.mult)
                nc.gpsimd.tensor_tensor_scan(
                    s_cur[:, HALF:], d0[:, HALF:], mmask[:, HALF:], 0.0,
                    AOP.add, AOP.mult)
            else:
                nc.vector.tensor_tensor_scan(s_cur[:], d0[:], mmask[:], 0.0,
                                             AOP.add, AOP.mult)
            # write back cols 1..31 (col 32 is the zeroed reset slot, col 0
            # is unchanged orig) — on ACT; the 8-deep s rotation absorbs
            # ACT FIFO bursts before the WAR on this tile bites
            nc.scalar.mul(
                buf3[:, :, qi + 1:qi + 32],
                s_cur[:].rearrange("p (g c) -> p g c", g=NBG)[:, :, 1:32],
                1.0)
            s_prev_box[0] = s_cur

        def emit_conv(b, gi0, ngi):
            # interior patch rows gi0..gi0+ngi-1 (= grid rows gi0+1..gi0+ngi)
            off = b * 1024 + gi0 * 32
            n = ngi * 32
            for m in range(CG):
                pt = ps.tile([128, n], F32, tag=f"ps{ngi}",
                             name=f"pt_{b}_{gi0}_{m}", bufs=3 if ngi == 4 else 4)
                for a in range(6):
                    nc.tensor.matmul(
                        pt[:],
                        lhsT=wt[:, a, 128 * m:128 * (m + 1)],
                        rhs=xpt[:, a, off:off + n],
                        start=(a == 0), stop=(a == 5))
                dst = buf4[:, b * CG + m, 1 + gi0:1 + gi0 + ngi, 1:33]
                nc.scalar.activation(
                    dst, pt[:].rearrange("p (gi gj) -> p gi gj", gi=ngi),
                    IDENT, bias=biast[:, m:m + 1])

        def emit_out_chunk(lo, hi):
            nc.sync.dma_start(
                xfr[:, :, :, lo:hi],
                buf3[:, :, lo:hi].rearrange("p (b g) q -> p b g q", b=B_LOC))

        chunk_after = {r: (lo, hi) for lo, hi, r in CHUNKS}
        # conv parts per piece: (gi0, ngi); pieces 0/1 are octet-0 quads
        CONV_PARTS = [(0, 4), (4, 4), (8, 8), (16, 8), (24, 8)]
        for piece, (gi0, ngi) in enumerate(CONV_PARTS):
            emit_conv(0, gi0, ngi)
            emit_conv(1, gi0, ngi)
            emit_band(piece)
            if piece == 4:
                emit_out_chunk(*chunk_after[-3])
            i0, i1 = PIECES[piece]
            for i in range(i0, i1 + 1):
                emit_row(i)
                if i in chunk_after:
                    emit_out_chunk(*chunk_after[i])

    _split_sp_multiwaits(nc)
    return nc


_NC = None


def kernel(x: np.ndarray, w: np.ndarray, b: np.ndarray) -> np.ndarray:
    global _NC, LAST_EXEC_NS
    B, C, H, _ = x.shape          # 16, 3, 512, 512
    assert (B, C, H) == (16, 3, 512)

    xp = x.reshape(B, 3, 32, 16, 32, 16)               # b c gi py gj px
    xp = np.ascontiguousarray(
        xp.transpose(1, 3, 5, 0, 2, 4)).reshape(768, B, 1024)
    xp = xp.astype(np.float16)
    wT = np.ascontiguousarray(w.reshape(768, 768).T).astype(np.float16)
    b = np.ascontiguousarray(b, dtype=np.float32)

    if _NC is None:
        _NC = _build()

    trace = _install_ntff_hook()
    in_maps = [{"xP": np.ascontiguousarray(xp[:, 2 * r:2 * r + 2, :]),
                "wT": wT, "bias": b} for r in range(N_CORES)]
    try:
        res = run_bass_kernel_spmd(_NC, in_maps, core_ids=list(range(N_CORES)),
                                   trace=trace)
    except Exception:
        if not trace:
            raise
        res = run_bass_kernel_spmd(_NC, in_maps, core_ids=list(range(N_CORES)),
                                   trace=False)
    LAST_EXEC_NS = res.exec_time_ns
    globals()['LAST_RESULT'] = res

    xf = np.concatenate([res.results[r]["xf"] for r in range(N_CORES)], axis=0)
    out = xf.reshape(B, 3, 544, 544)[:, :, 16:528, 16:528]
    return np.ascontiguousarray(out.astype(np.float32))


# revision 37
# speedup vs baseline: 1.0188x; 1.0188x over previous
"""Trainium2 Bass kernel for nn_C_Aggregation_24807731101830.

Patch-embed conv (stride 16 = kernel 16) + sequential Gauss-Seidel-like
index-update scan over a flattened 34x34 grid, batch-sharded over 8 cores.

v2 (fp16 + engine rebalance):
  - all on-chip data fp16 (DVE 2x tensor_tensor, half DMA traffic), conv
    matmuls on fp16 inputs (full PE rate), PSUM accumulation fp32.
  - row scan in multiply-form: state = (d0 + state) * M, M = [1, .125*31, 0]
    per 33-col segment, folding the reference's *1/8 into the scan.
  - buf holds ONLY original conv values; scan rows go to DRAM directly from
    the per-row scan tiles (30 small DMAs over 4 per-octet base-region DMAs
    on the same queue, so DRAM write order is FIFO-correct). No writeback.
  - each scan is split: segments 0-5 on DVE, 6-11 on GpSimd (parallel).
  - inputs DMA'd from the ACT DGE queue (outputs own the SP queue), with a
    small prefetch (bias, wT, both batches' first column-quarter) so the
    first conv octet starts ~3us in.
"""
import sys
import types
import numpy as np

import concourse.mybir as mybir
from concourse import bass, tile
from concourse.bass_utils import run_bass_kernel_spmd
from contextlib import ExitStack

F32 = mybir.dt.float32
F16 = mybir.dt.float16
AOP = mybir.AluOpType
IDENT = mybir.ActivationFunctionType.Identity

N_CORES = 8
B_LOC = 2            # batches per core
CG = 6               # channel groups of 128
NBG = B_LOC * CG     # 12 scan lane-groups
Q34 = 1156           # 34*34
QF = NBG * Q34       # buf free size per partition

# GpSimd/Pool only implements the TensorTensor opcode family; the
# TensorScalarPtr family (tensor_tensor_scan, tensor_scalar) fails the
# walrus per-engine ISA check, so those stay on DVE/ACT.
POOL_SCAN = False
POOL_BORDERS = False

LAST_EXEC_NS = None


def _install_ntff_hook():
    try:
        import trn_agent_boot.trn_boot as tb
        mod = types.ModuleType("antenv.axon_hooks")
        holder = [None]
        mod.set_axon_ntff_profile_hook = lambda h: holder.__setitem__(0, h)
        mod.get_axon_ntff_profile_hook = lambda: holder[0]
        sys.modules["antenv.axon_hooks"] = mod
        import antenv
        antenv.axon_hooks = mod
        mod.set_axon_ntff_profile_hook(
            tb._ntff_profile_via_ctypes('/opt/axon/libaxon_pjrt.so'))
        return True
    except Exception:
        return False


def _split_sp_multiwaits(nc):
    """walrus for gen3 rejects >1 sync-wait on several instruction structs
    (TPB_CTRL, S3_LW, ...); hoist extra waits onto single-wait NOPs placed
    just before, on the same engine queue (semantically equivalent)."""
    cnt = 0
    for f in nc.m.functions:
        for blk in f.blocks:
            insts = blk.instructions
            i = 0
            while i < len(insts):
                inst = insts[i]
                si = getattr(inst, 'sync_info', None)
                if (getattr(inst, 'engine', None) is not None
                        and si is not None and si.on_wait and len(si.on_wait) > 1):
                    waits = list(si.on_wait)
                    new = []
                    for w in waits[:-1]:
                        nop = mybir.InstNoOp(name=f"mwfix-{inst.name}-{cnt}",
                                             ins=[], outs=[])
                        cnt += 1
                        nop.engine = inst.engine
                        nop.sync_info = mybir.SyncInfo(on_wait=[w], on_update=[])
                        new.append(nop)
                    inst.sync_info = mybir.SyncInfo(
                        on_wait=[waits[-1]], on_update=list(si.on_update or []))
                    insts[i:i] = new
                    i += len(new)
                i += 1
    return cnt


# rows of the scan that become runnable after each conv part (a part
# scattering up to grid row G covers flat < 34(G+1)+33; row i reads taps
# up to flat 32i+65)
PIECES = [(1, 7), (8, 16), (17, 24), (25, 30)]
# output DMA chunks of buf, flat [lo, hi), each ready after the given row
# (row r+1 writes from flat 32(r+1)+1); the never-scanned tail [993,1156)
# goes out early, right after octet 3's scatter (marker row -3)
CHUNKS = [(0, 512, 15), (512, 737, 22), (993, 1156, -3),
          (737, 897, 27), (897, 961, 29), (961, 993, 30)]


def _build():
    nc = bass.Bass("TRN2", target_bir_lowering=False)
    xP_d = nc.declare_dram_parameter("xP", [768, B_LOC, 1024], F16, isOutput=False)
    wT_d = nc.declare_dram_parameter("wT", [768, 768], F16, isOutput=False)
    bias_d = nc.declare_dram_parameter("bias", [768], F32, isOutput=False)
    xf_d = nc.declare_dram_parameter("xf", [B_LOC, 768, Q34], F16, isOutput=True)
    xfr = xf_d.rearrange("b (g p) q -> p b g q", p=128)

    with tile.TileContext(nc) as tc, ExitStack() as ctx:
        sb = ctx.enter_context(tc.tile_pool(name="sb", bufs=1))
        ps = ctx.enter_context(tc.tile_pool(name="ps", bufs=4, space="PSUM"))
        pbpool = ctx.enter_context(tc.tile_pool(name="pb", bufs=2))

        # ---- preload the ACT function table before anything else (the
        #      first real ACTIVATE otherwise eats a 1.3us table load) ----
        scr = sb.tile([128, 8], F16, tag="scr")
        nc.scalar.activation(scr[:], scr[:], IDENT)

        # ---- input loads: bias/weights + batch-1 x on the ACT DGE queue,
        #      batch-0 x on SP; both batches' first quarter go out first ----
        biast = sb.tile([128, 6], F32, tag="bias")
        nc.scalar.dma_start(biast[:], bias_d.rearrange("(a p) -> p a", p=128))
        wt = sb.tile([128, 6, 768], F16, tag="wt")
        wTr = wT_d.rearrange("(a p) c -> p a c", p=128)
        for h in range(2):
            nc.scalar.dma_start(wt[:, 3 * h:3 * h + 3, :],
                                wTr[:, 3 * h:3 * h + 3, :])
        xpt = sb.tile([128, 6, B_LOC * 1024], F16, tag="xpt")
        xPr = xP_d.rearrange("(a p) b q -> p a b q", p=128)
        xpt4 = xpt[:].rearrange("p a (b q) -> p a b q", b=B_LOC)
        # first quarter split into 128-col eighths so the first conv quad's
        # columns land as early as possible
        for q0, q1 in [(0, 128), (128, 256), (256, 512), (512, 768),
                       (768, 1024)]:
            for b in range(B_LOC):
                eng = nc.sync if b == 0 else nc.scalar
                eng.dma_start(xpt4[:, :, b:b + 1, q0:q1],
                              xPr[:, :, b:b + 1, q0:q1])

        # ---- constants ----
        mmask = sb.tile([128, NBG * 33], F16, tag="mmask")
        nc.vector.memset(mmask[:], 0.125)
        mm3 = mmask[:].rearrange("p (g c) -> p g c", g=NBG)
        nc.vector.memset(mm3[:, :, 0:1], 1.0)
        nc.vector.memset(mm3[:, :, 32:33], 0.0)
        zt = sb.tile([128, 64], F16, tag="zt")
        nc.vector.memset(zt[:], 0.0)



        # ---- buf: original conv values only, f = bg*1156 + flat ----
        buf = sb.tile([128, QF], F16, tag="buf")
        buf3 = buf[:].rearrange("p (bg q) -> p bg q", bg=NBG)
        buf4 = buf[:].rearrange("p (bg gi gj) -> p bg gi gj", bg=NBG, gi=34)

        # ---- scratch tiles for the scan chain (permanent; the chain is
        #      sequential so reuse is safe, s rotates 4-deep for the DMAs) ----
        ua = sb.tile([128, NBG * 31], F16, tag="ua")
        uav = ua[:].rearrange("p (g c) -> p g c", g=NBG)
        ub = sb.tile([128, NBG * 31], F16, tag="ub")
        ubv = ub[:].rearrange("p (g c) -> p g c", g=NBG)
        d0t = [sb.tile([128, NBG * 33], F16, tag=f"d0_{k}", name=f"d0_{k}")
               for k in range(2)]
        st = [sb.tile([128, NBG * 33], F16, tag=f"s_{k}", name=f"s_{k}")
              for k in range(8)]
        for k in range(2):   # reset slots must be finite once and forever
            nc.vector.memset(
                d0t[k][:].rearrange("p (g c) -> p g c", g=NBG)[:, :, 32:33], 0.0)

        # ---- borders = bias. On DVE (idle during the fill anyway): on ACT
        #      they clog the FIFO ahead of the first scatters ----
        for b in range(B_LOC):
            for m in range(CG):
                bg = b * CG + m
                bcol = biast[:, m:m + 1]
                views = [buf3[:, bg, 0:35],
                         buf3[:, bg:bg + 1, 67:67 + 34 * 31].rearrange(
                             "p o (r t) -> p (o r) t", t=34)[:, :, 0:2],
                         buf3[:, bg, 1121:1156]]
                zins = [zt[:, 0:35],
                        zt[:, 0:62].rearrange("p (r t) -> p r t", t=2),
                        zt[:, 0:35]]
                for v, z in zip(views, zins):
                    nc.vector.tensor_scalar(v, z, bcol, None, op0=AOP.add)

        def band_view(base, nrows):
            return buf3[:, :, base:base + 32 * nrows].rearrange(
                "p g (r t) -> p r g t", t=32)[:, :, :, 0:31]

        pb_piece = {}

        def emit_band(piece):
            """UNSCALED 4-tap of orig for rows i0..i1 (+ col-31 fix: the
            j=31 3-tap tap orig(flat 32i) that the zeroed s_prev col 32
            cannot supply)."""
            i0, i1 = PIECES[piece]
            nr = i1 - i0 + 1
            base = 32 * i0 + 2
            pb = pbpool.tile([128, nr * NBG * 31], F16, tag="PB",
                             name=f"pb_{piece}")
            pb4 = pb[:].rearrange("p (r g j) -> p r g j", r=nr, g=NBG)
            eng0 = nc.vector if piece == 0 else nc.gpsimd  # piece 0 is fill
            eng0.tensor_tensor(pb4, band_view(base, nr),
                               band_view(base + 30, nr), AOP.add)
            tmp = pbpool.tile([128, nr * NBG * 31], F16, tag="PTMP",
                              name=f"ptmp_{piece}")
            tmp4 = tmp[:].rearrange("p (r g j) -> p r g j", r=nr, g=NBG)
            nc.vector.tensor_tensor(tmp4, band_view(base + 31, nr),
                                    band_view(base + 32, nr), AOP.add)
            nc.vector.tensor_tensor(pb[:], pb[:], tmp[:], AOP.add)
            f0 = i0 if i0 > 1 else 2
            if f0 <= i1:
                nfix = i1 - f0 + 1
                fix_dst = pb4[:, f0 - i0:, :, 30:31]
                fix_src = buf3[:, :, 32 * f0:32 * f0 + 32 * nfix].rearrange(
                    "p g (r t) -> p r g t", t=32)[:, :, :, 0:1]
                nc.vector.scalar_tensor_tensor(
                    fix_dst, fix_src, 1.0, fix_dst, AOP.mult, AOP.add)
            pb_piece[piece] = (pb, i0)

        s_prev_box = [None]
        HALF = NBG * 33 // 2

        def emit_row(i):
            qi = 32 * i
            piece = next(p for p, (a, b) in enumerate(PIECES) if a <= i <= b)
            pb, i0 = pb_piece[piece]
            pbr = pb[:].rearrange("p (r g j) -> p r g j",
                                  r=PIECES[piece][1] - i0 + 1,
                                  g=NBG)[:, i - i0, :, :]
            if s_prev_box[0] is None:
                sp = buf3[:, :, 0:33]          # row 0 = orig
            else:
                sp = s_prev_box[0][:].rearrange("p (g c) -> p g c", g=NBG)
            # d0[j] = sp[j-1] + sp[j] + sp[j+1] + P4[j]   (j = 1..31)
            nc.vector.tensor_tensor(uav, sp[:, :, 0:31], sp[:, :, 2:33],
                                    AOP.add)
            nc.vector.tensor_tensor(ubv, sp[:, :, 1:32], pbr, AOP.add)
            d0 = d0t[i % 2]
            d3 = d0[:].rearrange("p (g c) -> p g c", g=NBG)
            nc.vector.tensor_tensor(d3[:, :, 1:32], uav, ubv, AOP.add)
            # col 0 seeds the segment (M=1); col 32 stays 0 from init.
            # The seed stays on DVE: on ACT it queues behind octet scatter
            # bursts and stalls the scan chain at piece boundaries.
            nc.vector.tensor_scalar(d3[:, :, 0:1], buf3[:, :, qi:qi + 1],
                                    1.0, None, op0=AOP.mult)
            s_cur = st[i % 8]
            if POOL_SCAN:
                nc.vector.tensor_tensor_scan(
                    s_cur[:, 0:HALF], d0[:, 0:HALF], mmask[:, 0:HALF], 0.0,
                    AOP.add, AOP.mult)
                nc.gpsimd.tensor_tensor_scan(
                    s_cur[:, HALF:], d0[:, HALF:], mmask[:, HALF:], 0.0,
                    AOP.add, AOP.mult)
            else:
                nc.vector.tensor_tensor_scan(s_cur[:], d0[:], mmask[:], 0.0,
                                             AOP.add, AOP.mult)
            # write back cols 1..31 (col 32 is the zeroed reset slot, col 0
            # is unchanged orig) — on ACT; the 8-deep s rotation absorbs
            # ACT FIFO bursts before the WAR on this tile bites
            nc.scalar.mul(
                buf3[:, :, qi + 1:qi + 32],
                s_cur[:].rearrange("p (g c) -> p g c", g=NBG)[:, :, 1:32],
                1.0)
            s_prev_box[0] = s_cur

        def emit_conv(b, gi0, ngi):
            # interior patch rows gi0..gi0+ngi-1 (= grid rows gi0+1..gi0+ngi)
            off = b * 1024 + gi0 * 32
            n = ngi * 32
            for m in range(CG):
                pt = ps.tile([128, n], F32, tag=f"ps{ngi}",
                             name=f"pt_{b}_{gi0}_{m}", bufs=3 if ngi == 4 else 4)
                for a in range(6):
                    nc.tensor.matmul(
                        pt[:],
                        lhsT=wt[:, a, 128 * m:128 * (m + 1)],
                        rhs=xpt[:, a, off:off + n],
                        start=(a == 0), stop=(a == 5))
                dst = buf4[:, b * CG + m, 1 + gi0:1 + gi0 + ngi, 1:33]
                nc.scalar.activation(
                    dst, pt[:].rearrange("p (gi gj) -> p gi gj", gi=ngi),
                    IDENT, bias=biast[:, m:m + 1])

        def emit_out_chunk(lo, hi):
            nc.sync.dma_start(
                xfr[:, :, :, lo:hi],
                buf3[:, :, lo:hi].rearrange("p (b g) q -> p b g q", b=B_LOC))

        chunk_after = {r: (lo, hi) for lo, hi, r in CHUNKS}
        CONV_PARTS = [(0, 8), (8, 8), (16, 8), (24, 8)]
        for piece, (gi0, ngi) in enumerate(CONV_PARTS):
            emit_conv(0, gi0, ngi)
            emit_conv(1, gi0, ngi)
            emit_band(piece)
            if piece == 3:
                emit_out_chunk(*chunk_after[-3])
            i0, i1 = PIECES[piece]
            for i in range(i0, i1 + 1):
                emit_row(i)
                if i in chunk_after:
                    emit_out_chunk(*chunk_after[i])

    _split_sp_multiwaits(nc)
    return nc


_NC = None


def kernel(x: np.ndarray, w: np.ndarray, b: np.ndarray) -> np.ndarray:
    global _NC, LAST_EXEC_NS
    B, C, H, _ = x.shape          # 16, 3, 512, 512
    assert (B, C, H) == (16, 3, 512)

    xp = x.reshape(B, 3, 32, 16, 32, 16)               # b c gi py gj px
    xp = np.ascontiguousarray(
        xp.transpose(1, 3, 5, 0, 2, 4)).reshape(768, B, 1024)
    xp = xp.astype(np.float16)
    wT = np.ascontiguousarray(w.reshape(768, 768).T).astype(np.float16)
    b = np.ascontiguousarray(b, dtype=np.float32)

    if _NC is None:
        _NC = _build()

    trace = _install_ntff_hook()
    in_maps = [{"xP": np.ascontiguousarray(xp[:, 2 * r:2 * r + 2, :]),
                "wT": wT, "bias": b} for r in range(N_CORES)]
    try:
        res = run_bass_kernel_spmd(_NC, in_maps, core_ids=list(range(N_CORES)),
                                   trace=trace)
    except Exception:
        if not trace:
            raise
        res = run_bass_kernel_spmd(_NC, in_maps, core_ids=list(range(N_CORES)),
                                   trace=False)
    LAST_EXEC_NS = res.exec_time_ns
    globals()['LAST_RESULT'] = res

    xf = np.concatenate([res.results[r]["xf"] for r in range(N_CORES)], axis=0)
    out = xf.reshape(B, 3, 544, 544)[:, :, 16:528, 16:528]
    return np.ascontiguousarray(out.astype(np.float32))
